# revision 28
# baseline (speedup 1.0000x reference)
"""GCNConvNet on 8 Trainium2 NeuronCores (Bass/Tile SPMD kernel).

Strategy (graph/data parallel, per sharding hint):
  - Nodes are relabeled on the host and sharded across 8 cores (balanced by
    in-degree).  Each core owns a contiguous range of new node ids and
    computes the conv stack for exactly those destination nodes.
  - Per layer: each core computes Z = H_shard @ W for its nodes (bf16).  The
    local z rows are split into 4 row-quarters; as soon as a quarter is
    complete an AllGather of that quarter starts, so the collectives overlap
    the remaining compute of the layer (a monolithic AllGather stalled all
    engines for ~110us per layer).  The AG outputs (zfullQ, one per quarter)
    are double-buffered by layer parity to avoid WAR stalls.
  - The sparse aggregation sum_{e: dst in shard} norm_e * Z[src_e] is done by
    (a) bulk dma_gather of Z rows in edge order (grouped into 128-edge blocks)
    (b) one PE matmul per block:  psum[feat, dst_span] += M_block^T @ S_block
        where S_block[slot, dst_in_group] = norm_e (host-precomputed), which
        applies the GCN edge normalization and the segment-sum in one op.
  - Everything on the z path is bf16 (z table, gathered blocks, S, Sself, W,
    h activations); accumulation stays f32 in PSUM.  bf16 halves every DMA
    byte and enables the PE Fast-Weight-Load path.
  - S, Sself and the gather indices are layer-invariant: loaded into SBUF
    once and reused by all 4 conv layers.
  - Bias + ReLU are fused into the PSUM->SBUF activation.  The aggregated
    tile H^T [feat, dst] is directly the stationary operand of the next
    layer's Z matmul - no transposes anywhere.
  - gather indices are int16 (hardware limit 32767), so the gather chunk ==
    the row-quarter (25200 rows); edges are bucketed by (dst-group,
    src-quarter) with each bucket padded to one 128-slot block.  A host-side
    4-dimensional balanced grouping of dsts keeps every bucket <= 128 edges.

kernel(**inputs) takes the FULL inputs and returns the FULL [N,1] output.
"""

import math
import numpy as np
import ml_dtypes

import concourse.bass as bass
import concourse.bacc as bacc
import concourse.tile as tile
import concourse.mybir as mybir
from concourse.bass_utils import run_bass_kernel_spmd

F32 = mybir.dt.float32
BF16 = mybir.dt.bfloat16
I16 = mybir.dt.int16
AF = mybir.ActivationFunctionType
NP_BF16 = ml_dtypes.bfloat16

CORES = 8
CHUNKS = 4  # == row-quarters of the z table
P = 128     # feature dim == partition dim
SINGLE_PACKET = False
OVERLAP_AG = True  # trigger quarter-AGs mid-layer (True) or at layer end
PREF = 2  # tiles of chunk-0..2 gathers hoisted above the last AG trigger


class Cfg:
    def __init__(self, n_nodes, g=30, tgp=17):
        assert n_nodes % (CORES * CHUNKS) == 0
        self.N = n_nodes
        self.NPC = n_nodes // CORES          # real nodes per core
        self.G = g                           # dsts per group (psum span)
        self.TGP = tgp                       # groups per tile
        self.DT = g * tgp                    # dsts per tile (<=512 psum bank)
        assert self.DT <= 512
        # groups padded so the local id space splits into 4 equal quarters
        self.NGROUPS = ((math.ceil(self.NPC / g) + CHUNKS - 1)
                        // CHUNKS) * CHUNKS
        self.QG = self.NGROUPS // CHUNKS     # groups per quarter
        self.NPCP = self.NGROUPS * g         # padded local id space
        self.QROWS = self.NPCP // CHUNKS     # local rows per quarter
        self.NQR = self.QROWS * CORES        # rows per zfullQ chunk
        assert self.NQR <= 32767             # int16 gather index limit
        self.NP = self.NPCP * CORES
        self.NPCQ = self.NPC // CHUNKS       # real nodes per (core, quarter)
        self.T = math.ceil(self.NGROUPS / tgp)
        self.tile_groups = [
            min(tgp, self.NGROUPS - t * tgp) for t in range(self.T)
        ]
        self.tile_dsts = [ng * g for ng in self.tile_groups]
        self.tile_slots = [ng * 128 for ng in self.tile_groups]
        self.idx_cols = [s // 16 for s in self.tile_slots]   # per chunk
        self.scols = [CHUNKS * ng * g for ng in self.tile_groups]
        self.idx_total = sum(c * CHUNKS for c in self.idx_cols)
        self.s_total = sum(self.scols)
        # self-loop diagonal blocks (streamed from zbuf, no gather)
        self.self_blocks = [math.ceil(dt / 128) for dt in self.tile_dsts]
        self.sself_cols = [nb * 128 for nb in self.self_blocks]
        self.sself_total = sum(self.sself_cols)
        # first tile whose z-emit completes quarter c
        self.q_done_tile = [
            math.ceil(self.QG * (c + 1) / tgp) - 1 for c in range(CHUNKS)
        ]
        # quarters 0..2 trigger their AG as soon as complete; quarter 3's AG
        # is triggered at the layer boundary, after the next layer's first
        # chunk-0..2 gathers are emitted (all collectives share one
        # completion-semaphore counter, so any consumer emitted after a
        # trigger waits for it — emission order IS the dependency).
        if OVERLAP_AG:
            # +2 tiles of slack so the trigger's input wait (the quarter's
            # last z write landing) doesn't block the gather stream
            self.trig_tile = [
                min(t + 2, self.T - 1) for t in self.q_done_tile[:CHUNKS - 1]
            ]
        else:
            self.trig_tile = [self.T - 1] * (CHUNKS - 1)


# ---------------------------------------------------------------------------
# host preprocessing
# ---------------------------------------------------------------------------

def _group_greedy(dvec, n_groups, gsize, cap=128):
    """Assign len(dvec) items into n_groups of <=gsize items each,
    keeping every per-chunk (4-dim) load <= cap.  dvec: [n,4] int."""
    n = dvec.shape[0]
    order = np.argsort(-dvec.sum(axis=1), kind="stable")
    loads = np.zeros((n_groups, CHUNKS), np.int64)
    sizes = np.zeros(n_groups, np.int64)
    group_of = np.empty(n, np.int64)
    for it, i in enumerate(order):
        cand = np.max(loads + dvec[i], axis=1).astype(np.float64)
        cand[sizes >= gsize] = np.inf
        # keep group sizes growing in lockstep so late (small) items always
        # have several candidate groups to choose from
        lim = sizes.min() + 2
        cand[sizes >= lim] = np.inf
        g = int(np.argmin(cand))
        group_of[i] = g
        loads[g] += dvec[i]
        sizes[g] += 1
    # repair pass: improving swaps until all chunk loads <= cap
    stall = 0
    for _ in range(60000):
        gbad, cbad = np.unravel_index(np.argmax(loads), loads.shape)
        worst = loads[gbad, cbad]
        if worst <= cap or stall > 40:
            break
        members = np.flatnonzero(group_of == gbad)
        others = np.flatnonzero(group_of != gbad)
        do = dvec[others]
        improved = False
        # consider the few largest contributors to the overloaded chunk
        for i in members[np.argsort(-dvec[members, cbad])[:4]]:
            di = dvec[i]
            base_g = loads[gbad] - di
            cand_g = np.max(base_g + do, axis=1)
            base_o = loads[group_of[others]] - do
            cand_o = np.max(base_o + di, axis=1)
            score = np.maximum(cand_g, cand_o)
            j = others[int(np.argmin(score))]
            if score.min() < worst:
                gj = group_of[j]
                loads[gbad] += dvec[j] - di
                loads[gj] += di - dvec[j]
                group_of[i] = gj
                group_of[j] = gbad
                improved = True
                break
        stall = 0 if improved else stall + 1
    return group_of, loads


def preprocess(x, edge_index, cfg: Cfg):
    N = cfg.N
    src_o = np.asarray(edge_index[0], np.int64)
    dst_o = np.asarray(edge_index[1], np.int64)

    deg = np.bincount(dst_o, minlength=N).astype(np.float64) + 1.0
    dinv = (1.0 / np.sqrt(deg)).astype(np.float64)

    # self loops are handled separately (streamed from the core's own z
    # rows), so the gathered edge stream holds only the real edges
    srcs = src_o
    dsts = dst_o
    norms = (dinv[srcs] * dinv[dsts]).astype(np.float32)
    norm_self = (dinv * dinv).astype(np.float32)

    # --- core assignment: snake over degree-sorted nodes (balances degree) ---
    order = np.argsort(-deg, kind="stable")
    pattern = np.concatenate([np.arange(CORES), np.arange(CORES)[::-1]])
    reps = math.ceil(N / (2 * CORES))
    core_seq = np.tile(pattern, reps)[:N]
    core_of = np.empty(N, np.int64)
    core_of[order] = core_seq
    counts = np.bincount(core_of, minlength=CORES)
    assert (counts == cfg.NPC).all(), counts

    # --- quarter assignment within each core (snake by degree again) ---
    quarter_of = np.empty(N, np.int64)
    qpat = np.concatenate([np.arange(CHUNKS), np.arange(CHUNKS)[::-1]])
    for k in range(CORES):
        nodes_k = np.flatnonzero(core_of == k)
        order_k = np.argsort(-deg[nodes_k], kind="stable")
        seq = np.tile(qpat, math.ceil(len(nodes_k) / (2 * CHUNKS)))
        quarter_of[nodes_k[order_k]] = seq[:len(nodes_k)]
    for k in range(CORES):
        qc = np.bincount(quarter_of[core_of == k], minlength=CHUNKS)
        assert (qc == cfg.NPCQ).all(), qc

    # --- per-dst chunk-degree vectors (chunk == src quarter) ---
    dvec = np.zeros((N, CHUNKS), np.int64)
    np.add.at(dvec, (dsts, quarter_of[srcs]), 1)

    # --- within-(core, quarter) grouping (4-dim balanced) ---
    local_of = np.empty(N, np.int64)
    for k in range(CORES):
        for q in range(CHUNKS):
            nodes_kq = np.flatnonzero((core_of == k) & (quarter_of == q))
            gof, loads = _group_greedy(dvec[nodes_kq], cfg.QG, cfg.G)
            assert loads.max() <= 128, (
                f"group chunk load {loads.max()} exceeds 128; lower cfg.G"
            )
            o = np.argsort(gof, kind="stable")
            gsorted = gof[o]
            first = np.r_[0, np.flatnonzero(np.diff(gsorted)) + 1]
            fo = np.zeros(cfg.QG, np.int64)
            fo[gsorted[first]] = first
            rank_in_group = np.arange(len(o)) - fo[gsorted]
            local_of[nodes_kq[o]] = (
                (q * cfg.QG + gsorted) * cfg.G + rank_in_group)
    new_of = core_of * cfg.NPCP + local_of
    newpos_of_old = new_of.copy()

    # --- edge bucket packing ---
    e_dst = new_of[dsts]
    e_src = new_of[srcs]
    e_core = e_dst // cfg.NPCP
    e_ldst = e_dst % cfg.NPCP
    e_g = e_ldst // cfg.G            # global group within core
    e_pos = e_ldst % cfg.G
    e_t = e_g // cfg.TGP
    e_gt = e_g % cfg.TGP
    e_srcl = e_src % cfg.NPCP
    e_chunk = e_srcl // cfg.QROWS    # src quarter
    e_lsrc = (e_src // cfg.NPCP) * cfg.QROWS + (e_srcl % cfg.QROWS)

    bucket = ((e_core * cfg.NGROUPS + e_g) * CHUNKS) + e_chunk
    # secondary sort by src row: each DMA engine then reads ascending HBM
    # addresses within a bucket (better row-buffer locality)
    so = np.lexsort((e_lsrc, bucket))
    sb = bucket[so]
    starts = np.r_[0, np.flatnonzero(np.diff(sb)) + 1]
    uniq = sb[starts]
    sizes = np.diff(np.r_[starts, len(sb)])
    assert sizes.max() <= 128, f"bucket overflow: {sizes.max()}"
    start_of = np.zeros(cfg.NGROUPS * CORES * CHUNKS, np.int64)
    start_of[uniq] = starts
    rank = np.arange(len(sb)) - start_of[sb]

    # idx / S arrays.  Padding slots must point at *valid* rows (their S
    # value is 0 so they contribute nothing); spread them across the chunk so
    # the padded reads don't hot-spot a single HBM row.
    rng_pad = np.random.default_rng(12345)
    idx_all = rng_pad.integers(0, cfg.NQR, (CORES, 16, cfg.idx_total),
                               dtype=np.int16)
    s_all = np.zeros((CORES, P, cfg.s_total), NP_BF16)
    idx_base = np.zeros(cfg.T, np.int64)
    s_base = np.zeros(cfg.T, np.int64)
    for t in range(1, cfg.T):
        idx_base[t] = idx_base[t - 1] + cfg.idx_cols[t - 1] * CHUNKS
        s_base[t] = s_base[t - 1] + cfg.scols[t - 1]

    r = rank  # slot-in-block for each sorted edge
    core_s = e_core[so]
    t_s = e_t[so]
    gt_s = e_gt[so]
    c_s = e_chunk[so]
    pos_s = e_pos[so]
    lsrc_s = e_lsrc[so]
    norm_s = norms[so]

    slot = gt_s * 128 + r
    icol = idx_base[t_s] + c_s * np.array(cfg.idx_cols)[t_s] + (slot // 16)
    irow = slot % 16
    idx_all[core_s, irow, icol] = lsrc_s.astype(np.int16)

    ngt = np.array(cfg.tile_groups)[t_s]
    scol = s_base[t_s] + (c_s * ngt + gt_s) * cfg.G + pos_s
    s_all[core_s, r, scol] = norm_s.astype(NP_BF16)

    # S_self: per-tile diagonal blocks scaling the core's own z rows
    sself_base = np.zeros(cfg.T, np.int64)
    for t in range(1, cfg.T):
        sself_base[t] = sself_base[t - 1] + cfg.sself_cols[t - 1]
    sself_all = np.zeros((CORES, P, cfg.sself_total), NP_BF16)
    dtile = cfg.G * cfg.TGP
    n_core = new_of // cfg.NPCP
    n_local = new_of % cfg.NPCP
    n_t = (n_local // cfg.G) // cfg.TGP
    n_pos = n_local - n_t * dtile
    sself_all[n_core, n_pos % 128, sself_base[n_t] + n_pos] = (
        norm_self.astype(NP_BF16))

    # x shards, feature-major, zero-padded at hole ids
    xT_all = np.zeros((CORES, P, cfg.NPCP), NP_BF16)
    xT_all[n_core, :, n_local] = np.asarray(x, np.float32).astype(NP_BF16)

    return dict(idx_all=idx_all, s_all=s_all, sself_all=sself_all,
                xT_all=xT_all, newpos_of_old=newpos_of_old)


# ---------------------------------------------------------------------------
# bass kernel
# ---------------------------------------------------------------------------

def build_nc(cfg: Cfg):
    nc = bacc.Bacc("TRN2", target_bir_lowering=False, debug=False,
                   num_devices=CORES, num_swdge_queues=4)

    xT = nc.dram_tensor("xT", [P, cfg.NPCP], BF16, kind="ExternalInput")
    idxd = nc.dram_tensor("idx", [16, cfg.idx_total], I16, kind="ExternalInput")
    sd = nc.dram_tensor("S", [P, cfg.s_total], BF16, kind="ExternalInput")
    ssd = nc.dram_tensor("Sself", [P, cfg.sself_total], BF16,
                         kind="ExternalInput")
    wd = nc.dram_tensor("W", [P, 4 * P], BF16, kind="ExternalInput")
    bd = nc.dram_tensor("B", [P, 4], F32, kind="ExternalInput")
    lw1d = nc.dram_tensor("lw1", [P, 64], BF16, kind="ExternalInput")
    lb1d = nc.dram_tensor("lb1", [64, 1], F32, kind="ExternalInput")
    lw2d = nc.dram_tensor("lw2", [64, 1], BF16, kind="ExternalInput")
    lb2d = nc.dram_tensor("lb2", [1, 1], F32, kind="ExternalInput")
    outd = nc.dram_tensor("out", [cfg.NPCP, 1], F32, kind="ExternalOutput")

    # local z rows, one tensor per quarter (separate tensors give the
    # dependency tracker exact AG-input granularity)
    zbufQ = [nc.dram_tensor(f"zbufQ{c}", [cfg.QROWS, P], BF16)
             for c in range(CHUNKS)]
    # AllGather outputs, double-buffered by z-generation parity
    zfullQ = [[nc.dram_tensor(f"zfullQ{p}_{c}", [cfg.NQR, P], BF16,
                              addr_space="Shared")
               for c in range(CHUNKS)] for p in range(2)]

    idx_base = [0]
    s_base = [0]
    ss_base = [0]
    for t in range(1, cfg.T):
        idx_base.append(idx_base[-1] + cfg.idx_cols[t - 1] * CHUNKS)
        s_base.append(s_base[-1] + cfg.scols[t - 1])
        ss_base.append(ss_base[-1] + cfg.sself_cols[t - 1])

    with tile.TileContext(nc) as tc:
        with tc.tile_pool(name="const", bufs=1) as cp, \
             tc.tile_pool(name="sb", bufs=2) as sbp, \
             tc.tile_pool(name="mpool", bufs=10) as mp, \
             tc.tile_pool(name="psagg", bufs=3, space="PSUM") as pp_agg, \
             tc.tile_pool(name="psz", bufs=2, space="PSUM") as pp_z, \
             tc.tile_pool(name="pshead", bufs=1, space="PSUM") as pp_head:

            # gather indices first: the hoisted layer-0 gathers need them
            idx_sb = cp.tile([P, cfg.idx_total], I16)
            for q in range(8):
                nc.sync.dma_start(idx_sb[16 * q:16 * (q + 1), :], idxd[:, :])
            w_sb = cp.tile([P, 4 * P], BF16)
            nc.sync.dma_start(w_sb[:], wd[:, :])
            b_sb = cp.tile([P, 4], F32)
            nc.sync.dma_start(b_sb[:], bd[:, :])
            lw1_sb = cp.tile([P, 64], BF16)
            nc.sync.dma_start(lw1_sb[:], lw1d[:, :])
            lb1_sb = cp.tile([64, 1], F32)
            nc.sync.dma_start(lb1_sb[:], lb1d[:, :])
            lw2_sb = cp.tile([64, 1], BF16)
            nc.sync.dma_start(lw2_sb[:], lw2d[:, :])
            lb2_sb = cp.tile([1, 1], F32)
            nc.sync.dma_start(lb2_sb[:], lb2d[:, :])
            # big layer-invariant tables: allocated here, but their loads are
            # issued after the layer-0 z DMAs (below) so they don't serialize
            # the sync queue ahead of the prologue's x/z traffic; they land
            # while the first AllGathers are in flight.
            s_sb = cp.tile([P, cfg.s_total], BF16)
            ss_sb = cp.tile([P, cfg.sself_total], BF16)

            def zspan(engine, grow0, rows, sb_tile, write):
                """DMA sb_tile[0:rows,:] <-> zbuf global rows [grow0, ..),
                split at quarter boundaries."""
                a, off, rem = grow0, 0, rows
                while rem > 0:
                    q, qa = divmod(a, cfg.QROWS)
                    n = min(rem, cfg.QROWS - qa)
                    if write:
                        engine.dma_start(zbufQ[q][qa:qa + n, :],
                                         sb_tile[off:off + n, :])
                    else:
                        engine.dma_start(sb_tile[off:off + n, :],
                                         zbufQ[q][qa:qa + n, :])
                    a += n
                    off += n
                    rem -= n

            def emit_z(h_tile, layer, t):
                """z rows for tile t of layer `layer` (reads W[layer])."""
                dt = cfg.tile_dsts[t]
                r0 = t * cfg.DT
                for s0 in range(0, dt, P):
                    sl = min(P, dt - s0)
                    zp = pp_z.tile([P, P], F32, tag="zp",
                                   name=f"zp{layer}_{t}_{s0}")
                    nc.tensor.matmul(
                        zp[0:sl, :],
                        lhsT=h_tile[:, s0:s0 + sl],
                        rhs=w_sb[:, layer * P:(layer + 1) * P],
                        start=True, stop=True)
                    zs = sbp.tile([P, P], BF16, tag="zs",
                                  name=f"zs{layer}_{t}_{s0}")
                    nc.vector.tensor_copy(zs[0:sl, :], zp[0:sl, :])
                    zspan(nc.sync, r0 + s0, sl, zs, write=True)

            def ag(gen, c):
                nc.gpsimd.collective_compute(
                    "AllGather", mybir.AluOpType.bypass,
                    replica_groups=[list(range(CORES))],
                    ins=[zbufQ[c].ap()], outs=[zfullQ[gen % 2][c].ap()])

            def emit_head(h_tile, t):
                dt = cfg.tile_dsts[t]
                r0 = t * cfg.DT
                hp = pp_head.tile([64, cfg.DT], F32, tag="hp", name=f"hp{t}")
                nc.tensor.matmul(hp[:, 0:dt], lhsT=lw1_sb[:],
                                 rhs=h_tile[:, 0:dt], start=True, stop=True)
                ha = sbp.tile([64, cfg.DT], BF16, tag="ha", name=f"ha{t}")
                nc.scalar.activation(ha[:, 0:dt], hp[:, 0:dt], AF.Relu,
                                     bias=lb1_sb[:])
                op = pp_head.tile([1, cfg.DT], F32, tag="op", name=f"op{t}")
                nc.tensor.matmul(op[:, 0:dt], lhsT=lw2_sb[:],
                                 rhs=ha[0:64, 0:dt], start=True, stop=True)
                ob = sbp.tile([1, cfg.DT], F32, tag="ob", name=f"ob{t}")
                nc.scalar.activation(ob[:, 0:dt], op[:, 0:dt], AF.Sigmoid,
                                     bias=lb2_sb[:])
                nc.sync.dma_start(
                    outd[r0:r0 + dt, :].rearrange("a b -> b a"), ob[:, 0:dt])

            def gather_one(layer, t, c, par):
                ng = cfg.tile_groups[t]
                slots = cfg.tile_slots[t]
                m = mp.tile([P, cfg.TGP * P], BF16, tag="m",
                            name=f"m{layer}_{t}_{c}")
                m3 = m[:, 0:ng * P].rearrange("p (b e) -> p b e", e=P)
                ic0 = idx_base[t] + c * cfg.idx_cols[t]
                nc.gpsimd.dma_gather(
                    m3,
                    zfullQ[par][c][:, :],
                    idx_sb[:, ic0:ic0 + cfg.idx_cols[t]],
                    slots, slots, P, single_packet=SINGLE_PACKET,
                    queue_num=c)
                return m

            mcache = {}

            def boundary(layer):
                """Between z-emission of generation `layer` and its last AG:
                hoist the next aggregation's first chunk-0..2 gathers above
                the AG-3 trigger so they don't wait for it (shared collective
                semaphore counter == emission-order dependency)."""
                for t in range(min(PREF, cfg.T)):
                    for c in range(CHUNKS - 1):
                        mcache[(t, c)] = gather_one(layer, t, c, layer % 2)
                ag(layer, CHUNKS - 1)

            # ---- layer 0: z from x; AG each quarter as soon as complete ----
            for t in range(cfg.T):
                dt = cfg.tile_dsts[t]
                r0 = t * cfg.DT
                xt = sbp.tile([P, cfg.DT], BF16, tag="xt", name=f"xt{t}")
                nc.sync.dma_start(xt[:, 0:dt], xT[:, r0:r0 + dt])
                emit_z(xt, 0, t)
                for c in range(CHUNKS - 1):
                    if cfg.trig_tile[c] == t:
                        ag(0, c)

            # layer-invariant table loads (overlap the prologue AllGathers);
            # S split into column chunks so early tiles can start as soon as
            # their slice has landed (if dep tracking is range-precise)
            nsplit = 4
            step = ((cfg.s_total + nsplit - 1) // nsplit + 1) & ~1
            for a in range(0, cfg.s_total, step):
                b = min(a + step, cfg.s_total)
                nc.sync.dma_start(s_sb[:, a:b], sd[:, a:b])
            nc.sync.dma_start(ss_sb[:], ssd[:, :])

            boundary(0)

            # ---- conv layers ----
            for layer in range(4):
                last = layer == 3
                par = layer % 2
                for t in range(cfg.T):
                    ng = cfg.tile_groups[t]
                    dt = cfg.tile_dsts[t]
                    ms = []
                    for c in range(CHUNKS):
                        m = mcache.pop((t, c), None)
                        if m is None:
                            m = gather_one(layer, t, c, par)
                        ms.append(m)
                    ps = pp_agg.tile([P, cfg.DT], F32, tag="agg",
                                     name=f"agg{layer}_{t}")
                    k = 0
                    for c in range(CHUNKS):
                        for g in range(ng):
                            nc.tensor.matmul(
                                ps[:, g * cfg.G:(g + 1) * cfg.G],
                                lhsT=ms[c][:, g * P:(g + 1) * P],
                                rhs=s_sb[:, s_base[t] + (c * ng + g) * cfg.G:
                                         s_base[t] + (c * ng + g + 1) * cfg.G],
                                start=(k == 0), stop=False)
                            k += 1
                    # self-loop contribution: own z rows * diag(norm_self)
                    r0 = t * cfg.DT
                    nsb = cfg.self_blocks[t]
                    for b in range(nsb):
                        rows = min(P, dt - b * P)
                        zown = sbp.tile([P, P], BF16, tag="zown",
                                        name=f"zo{layer}_{t}_{b}")
                        zspan(nc.sync, r0 + b * P, rows, zown, write=False)
                        nc.tensor.matmul(
                            ps[:, b * P:b * P + rows],
                            lhsT=zown[0:rows, :],
                            rhs=ss_sb[0:rows, ss_base[t] + b * P:
                                      ss_base[t] + b * P + rows],
                            start=False, stop=(b == nsb - 1))
                    h = sbp.tile([P, cfg.DT], BF16, tag="h",
                                 name=f"h{layer}_{t}")
                    nc.scalar.activation(
                        h[:, 0:dt], ps[:, 0:dt],
                        AF.Relu if layer < 3 else AF.Identity,
                        bias=b_sb[:, layer:layer + 1])
                    if not last:
                        emit_z(h, layer + 1, t)
                        for c in range(CHUNKS - 1):
                            if cfg.trig_tile[c] == t:
                                ag(layer + 1, c)
                    else:
                        emit_head(h, t)
                if not last:
                    boundary(layer + 1)

    nc.compile()
    return nc


# ---------------------------------------------------------------------------
# entry point
# ---------------------------------------------------------------------------

_CACHE = {}


def _get_nc(cfg: Cfg):
    key = (cfg.N, cfg.G, cfg.TGP)
    if key not in _CACHE:
        _CACHE[key] = build_nc(cfg)
    return _CACHE[key]


def run(x, edge_index, w0, b0, w1, b1, w2, b2, w3, b3, lw1, lb1, lw2, lb2,
        cfg: Cfg):
    pre = preprocess(x, edge_index, cfg)
    W = np.concatenate([np.asarray(w, np.float32)
                        for w in (w0, w1, w2, w3)], axis=1).astype(NP_BF16)
    B = np.stack([np.asarray(b, np.float32)
                  for b in (b0, b1, b2, b3)], axis=1)        # [128, 4] f32
    in_maps = []
    for k in range(CORES):
        in_maps.append({
            "xT": pre["xT_all"][k],
            "idx": pre["idx_all"][k],
            "S": pre["s_all"][k],
            "Sself": pre["sself_all"][k],
            "W": W,
            "B": B,
            "lw1": np.asarray(lw1, np.float32).astype(NP_BF16),
            "lb1": np.asarray(lb1, np.float32).reshape(64, 1),
            "lw2": np.asarray(lw2, np.float32).astype(NP_BF16),
            "lb2": np.asarray(lb2, np.float32).reshape(1, 1),
        })
    nc = _get_nc(cfg)
    res = run_bass_kernel_spmd(nc, in_maps, core_ids=list(range(CORES)))
    out_new = np.concatenate([res.results[k]["out"] for k in range(CORES)],
                             axis=0)  # [NP, 1] in padded new-id order
    out = out_new[pre["newpos_of_old"]]
    return out, res


def make_cfg(n_nodes):
    return Cfg(n_nodes, g=30, tgp=17)


def kernel(x, edge_index, batch, w0, b0, w1, b1, w2, b2, w3, b3,
           lw1, lb1, lw2, lb2):
    x = np.asarray(x, np.float32)
    cfg = make_cfg(x.shape[0])
    out, _ = run(x, edge_index, w0, b0, w1, b1, w2, b2, w3, b3,
                 lw1, lb1, lw2, lb2, cfg)
    return out
